# revision 37
# baseline (speedup 1.0000x reference)
"""MoE top-2 routing kernel for 8 Trainium2 NeuronCores.

Strategy (expert-parallel with two-segment load balancing):
  - Host computes the (tiny) router in float64: logits -> softmax -> top-2 ->
    renormalize.  Selection was verified tie-safe: min prob gap between
    2nd/3rd expert is ~8e-6 while cross-backend fp32 logit noise is ~3e-7.
  - Plain expert-parallel pads every core to the max expert count (1072 for
    this input vs 1024 average).  Instead each core gets TWO fixed-size
    segments (a, b), each with its own expert weight set; the host assigns
    expert -> slot multisets: the 2 largest experts take two a-slots
    (2a >= c_max), the 2 smallest take two b-slots, the middle four take
    one a + one b (a+b >= c_mid).  For this input a=536, b=500: capacity
    1036/core instead of 1072 (-7.5us of matmul).  Cores stream 2 full
    weight sets (33.6MB, ~150GB/s — well under the ~310GB/s queue budget),
    and all token blocks stay >=232 columns so LDWEIGHTS stays hidden.
  - Each core runs a dense FFN per segment:
        hT = gelu(w1T.T-contractions)   (PSUM fp32 accum, bias fused in ACT)
        oT = w2-contractions over hT
    with D/H features on the partition axis end-to-end (no on-device
    transposes).  Host applies combine weights and scatter-adds outputs.

Schedule notes (from perfetto/ntff trace analysis):
  - The ramp is HBM-bandwidth-bound.  Queue FIFO order IS the schedule:
    sync carries a pure w1 stream in ho-PAIRS (4096B DMA rows; 2048B rows
    measured 2-3x slower per queue), scalar carries the x blocks with b1
    tucked between them (2 DMA queues total; a third gpsimd queue was
    dropped — the ~8.8us teardown epilogue measured identical without it).
  - Fetches not needed until late (B-segment x blocks, early w2 slabs) are
    emitted onto the SYNC queue mid-L1 so queue FIFO places them behind the
    w1 pair stream.  (A memzero-WAR "gate" does NOT work: the scheduler
    hoists the dep-free memzero itself, verified in the trace.)
  - L2 runs segment B first and A last, so the final serial eviction drains
    A's small 240-col block; A splits [296, 240] (not [304, 232]) because a
    232-col matmul (116cy) sits at the LDWEIGHTS boundary (~115cy) and
    jitters LDW-bound.
  - 13 dependency-free warm-up matmuls bridge the ~8.3us fixed prologue so
    the HAM clock-gate is at 8/8 when real matmuls start; steady state runs
    at the bf16 peak (2 cols/cycle).
  - Remaining fixed costs (framework): ~8.3us prologue before the first DMA
    packet, ~8.8us BSP semaphore-teardown epilogue.

Per-core layouts (D=1024, H=4096; cap = a+b tokens, A span [0,a), B [a,cap)):
  xT   [128, 8*cap]        bf16   block-major: xT[p, ko, t] = x_g[t, ko*128+p]
  w1A/B [128, 32, 8, 128]  bf16   w1s[p, ho, ko, j] = w1[e][ho*128+j, ko*128+p]
  w2A/B [128, 8, 32, 128]  bf16   w2s[p, do, ko, j] = w2[e][do*128+j, ko*128+p]
  b1A/B [128, 32]          f32    b1s[p, ho]        = b1[e][ho*128+p]
  oT   [128, 8, cap]       f32    oT[p, do, t]      = o_g[t, do*128+p]
"""

import numpy as np
import ml_dtypes

TOP_K = 2
P = 128
D = 1024
H = 4096
E = 8

_COMPILED = {}  # (a, b) or ('single', C) -> compiled Bacc instance


def _ceil8(n):
    return ((n + 7) // 8) * 8


def _split_seg(S, first):
    """Split segment of S tokens into blocks: `first`-sized lead block, the
    rest as even blocks <=512 (all >=232 when S allows, so LDWEIGHTS stays
    hidden behind the previous matmul)."""
    if S <= 512:
        return [S]
    b0 = min(first, S - 232)
    rem = S - b0
    nblk = -(-rem // 480)
    sizes = [b0]
    for i in range(nblk):
        s = -(-rem // (nblk - i))
        s = min(_ceil8(s), rem)
        sizes.append(s)
        rem -= s
    assert sum(sizes) == S and all(s <= 512 for s in sizes), sizes
    return sizes


def _seg_blocks(a, b):
    # 296/240 rather than 304/232: a 232-col matmul (116cy) sits exactly at
    # the LDWEIGHTS boundary (~115cy) and jitters LDW-bound; 240 has margin.
    return _split_seg(a, 296), _split_seg(b, 272)


def _build_dual_kernel(a, b):
    import concourse.mybir as mybir
    import concourse.tile as tile
    from concourse import bacc

    blocks_a, blocks_b = _seg_blocks(a, b)
    blocks = blocks_a + blocks_b
    seg_of = [0] * len(blocks_a) + [1] * len(blocks_b)
    starts = [sum(blocks[:i]) for i in range(len(blocks))]
    cap = a + b
    NTMAX = max(blocks)
    nb = len(blocks)
    nba = len(blocks_a)
    bf16 = mybir.dt.bfloat16
    f32 = mybir.dt.float32

    nc = bacc.Bacc("TRN2", target_bir_lowering=False, debug=False)
    xT = nc.dram_tensor("xT", [P, D // P * cap], bf16, kind="ExternalInput").ap()
    w1d = [
        nc.dram_tensor(n, [P, H // P, D // P, P], bf16, kind="ExternalInput").ap()
        for n in ("w1A", "w1B")
    ]
    w2d = [
        nc.dram_tensor(n, [P, D // P, H // P, P], bf16, kind="ExternalInput").ap()
        for n in ("w2A", "w2B")
    ]
    b1d = [
        nc.dram_tensor(n, [P, H // P], f32, kind="ExternalInput").ap()
        for n in ("b1A", "b1B")
    ]
    oT = nc.dram_tensor("oT", [P, D // P, cap], f32, kind="ExternalOutput").ap()

    with tile.TileContext(nc) as tc:
        with (
            tc.tile_pool(name="const", bufs=1) as cpool,
            tc.tile_pool(name="resident", bufs=1) as rpool,
            tc.tile_pool(name="warm", bufs=1) as warmpool,
            tc.tile_pool(name="w1p", bufs=6) as w1pool,
            tc.tile_pool(name="w2p", bufs=3) as w2pool,
            tc.tile_pool(name="ost", bufs=4) as opool,
            tc.tile_pool(name="ps", bufs=4, space="PSUM") as pspool,
            tc.tile_pool(name="wps", bufs=1, space="PSUM") as wpspool,
        ):
            # PE warm-up: dependency-free matmuls keep the HAM clock-gate at
            # 8/8 while the first input DMAs are in flight.
            wsrc = warmpool.tile([P, 512], bf16)
            nc.gpsimd.memset(wsrc[:], 0.0)
            wps = wpspool.tile([P, 512], f32)
            for _ in range(13):
                nc.tensor.matmul(wps[:], wsrc[:, :P], wsrc[:], start=True, stop=True)

            # b1 rides the scalar queue (between the x blocks) rather than a
            # dedicated gpsimd queue: one fewer DMA queue to tear down in the
            # fixed epilogue, and the tiny 128B-row descriptors would slow
            # the queue head if placed first.
            b1A_sb = cpool.tile([P, H // P], f32, tag="b1A")
            b1B_sb = cpool.tile([P, H // P], f32, tag="b1B")
            b1_sb = [b1A_sb, b1B_sb]

            w1_tiles = {}

            def fetch_w1(seg, ho):
                base = ho & ~1
                t = w1pool.tile([P, 2, D // P, P], bf16, tag="w1s")
                nc.sync.dma_start(t[:], w1d[seg][:, base : base + 2])
                w1_tiles[(seg, base)] = t
                w1_tiles[(seg, base + 1)] = t

            fetch_w1(0, 0)
            # A-segment x blocks fetch immediately on scalar (needed during
            # the ramp).  B-segment blocks and the early w2A slabs are NOT
            # needed until ~70us/~130us; fetching them early starves the w1
            # pair stream, so their descriptors are emitted onto the SYNC
            # queue mid-L1 — same-queue FIFO places them behind the w1 pairs.
            # (A memzero-WAR "gate" does NOT work: the scheduler hoists the
            # dep-free memzero itself, as seen in the trace.)
            x_blks = []
            for blk in range(nb):
                st, sz = starts[blk], blocks[blk]
                xb = rpool.tile([P, D // P * sz], bf16, tag=f"xb{blk}")
                if seg_of[blk] == 0:
                    nc.scalar.dma_start(xb[:], xT[:, D // P * st : D // P * (st + sz)])
                    if blk == 0:
                        # b1A behind xA0 (needed by the first ACT ~14.5us);
                        # b1B much later (first B ACT ~75us)
                        nc.scalar.dma_start(b1A_sb[:], b1d[0][:])
                x_blks.append(xb)
            nc.scalar.dma_start(b1B_sb[:], b1d[1][:])

            h_sb = rpool.tile([P, H // P, cap], bf16)

            def w1_src(seg, ho, ko):
                return w1_tiles[(seg, ho)][:, ho % 2, ko, :]

            def x_src(blk, ko):
                sz = blocks[blk]
                return x_blks[blk][:, ko * sz : (ko + 1) * sz]

            # Group order: lead = first 6 ho rows on block 0 (later x blocks
            # still in flight), then their remaining A blocks, then ho-major
            # over segment A, then ho-major over segment B (its w1 stream
            # arrives during A's compute).  Keeps slab lifetimes short and
            # slab demand well under delivery after the ramp.
            lead = min(6, H // P) if nba >= 2 else 0
            pairs = [(k, 0) for k in range(lead)]
            for k in range(lead):
                pairs += [(k, bi) for bi in range(1, nba)]
            for ho in range(lead, H // P):
                pairs += [(ho, bi) for bi in range(nba)]
            for ho in range(H // P):
                pairs += [(ho, bi) for bi in range(nba, nb)]

            # Layer 1: hT[:, ho, t] = gelu(sum_ko w1.T @ x + b1)
            w2_early = []
            for pi, (ho, blk) in enumerate(pairs):
                seg = seg_of[blk]
                if (seg, ho) not in w1_tiles:
                    fetch_w1(seg, ho)
                st, sz = starts[blk], blocks[blk]
                ps = pspool.tile([P, NTMAX], f32, tag="ps")
                for ko in range(D // P):
                    nc.tensor.matmul(
                        ps[:, :sz],
                        w1_src(seg, ho, ko),
                        x_src(blk, ko),
                        start=(ko == 0),
                        stop=(ko == D // P - 1),
                    )
                nc.scalar.activation(
                    h_sb[:, ho, st : st + sz],
                    ps[:, :sz],
                    mybir.ActivationFunctionType.Gelu,
                    bias=b1_sb[seg][:, ho : ho + 1],
                )
                if seg == 0 and blk == 0 and ho == 20:
                    # behind ~10 w1 pairs in sync-queue FIFO; arrives ~30us,
                    # needed at the B sweep (~70us)
                    for bi in range(nb):
                        if seg_of[bi] == 1:
                            sti, szi = starts[bi], blocks[bi]
                            nc.sync.dma_start(
                                x_blks[bi][:],
                                xT[:, D // P * sti : D // P * (sti + szi)],
                            )
                if seg == 0 and blk == 0 and ho in (22, 26, 30):
                    # early w2B slabs (L2 runs segment B first), behind 11-15
                    # w1 pairs in queue order; needed at L2 start (~130us)
                    w2s = w2pool.tile([P, H // P, P], bf16, tag="w2s")
                    nc.sync.dma_start(w2s[:], w2d[1][:, len(w2_early)])
                    w2_early.append(w2s)

            # Layer 2: oT[:, do, t] = sum_ko w2[:,do,ko,:].T @ hT[:,ko,t]
            # Segment B first, A last: the final eviction then drains A's
            # small 240-col block instead of B's 504-col one.
            seg_order = (1, 0)
            for si, seg in enumerate(seg_order):
                sblks = [i for i in range(nb) if seg_of[i] == seg]
                for do in range(D // P):
                    if seg == 1 and do < len(w2_early):
                        w2s = w2_early[do]
                    else:
                        w2s = w2pool.tile([P, H // P, P], bf16, tag="w2s")
                        eng = nc.sync if do % 2 == 0 else nc.scalar
                        eng.dma_start(w2s[:], w2d[seg][:, do])
                    for bi in sblks:
                        st, sz = starts[bi], blocks[bi]
                        ps = pspool.tile([P, NTMAX], f32, tag="ps")
                        for ko in range(H // P):
                            nc.tensor.matmul(
                                ps[:, :sz],
                                w2s[:, ko, :],
                                h_sb[:, ko, st : st + sz],
                                start=(ko == 0),
                                stop=(ko == H // P - 1),
                            )
                        last = (
                            si == len(seg_order) - 1
                            and do == D // P - 1
                            and bi == sblks[-1]
                        )
                        if not last:
                            ob = opool.tile([P, NTMAX], f32, tag="ob")
                            nc.vector.tensor_copy(ob[:, :sz], ps[:, :sz])
                            nc.scalar.dma_start(oT[:, do, st : st + sz], ob[:, :sz])
                        else:
                            # Final eviction is on the critical path: split it
                            # so the first half's DMA overlaps the second
                            # half's copy, using both queues.
                            hsz = sz // 2
                            ob = opool.tile([P, NTMAX], f32, tag="ob")
                            nc.vector.tensor_copy(ob[:, :hsz], ps[:, :hsz])
                            nc.sync.dma_start(oT[:, do, st : st + hsz], ob[:, :hsz])
                            nc.vector.tensor_copy(ob[:, hsz:sz], ps[:, hsz:sz])
                            nc.scalar.dma_start(
                                oT[:, do, st + hsz : st + sz], ob[:, hsz:sz]
                            )

    nc.compile()
    return nc


def _route_host(x_flat, router_w):
    """Float64 router: returns per-expert (token_idx, combine_weight)."""
    logits = x_flat.astype(np.float64) @ router_w.astype(np.float64).T
    m = logits.max(axis=-1, keepdims=True)
    p = np.exp(logits - m)
    p /= p.sum(axis=-1, keepdims=True)
    order = np.argsort(-p, axis=-1)
    topi = order[:, :TOP_K]
    topw = np.take_along_axis(p, topi, axis=-1)
    topw /= topw.sum(axis=-1, keepdims=True)

    idx_list, wgt_list = [], []
    for e in range(E):
        mask = topi == e  # [T, TOP_K]; at most one True per row
        rows = np.nonzero(mask.any(axis=-1))[0]
        w = topw[rows][mask[rows]]
        idx_list.append(rows)
        wgt_list.append(w.astype(np.float32))
    return idx_list, wgt_list


def _plan_slots(counts):
    """Two-segment balancing: returns (a, b, slots) where slots is a list of
    8 (expert_a, expert_b) core assignments, or None if not profitable.
    Each expert's tokens are later split greedily across its slots."""
    def _ceil4(n):
        return ((n + 3) // 4) * 4

    order = sorted(range(E), key=lambda e: -counts[e])
    big, mid, small = order[:2], order[2:-2], order[-2:]
    a = _ceil4(-(-max(counts[e] for e in big) // 2))
    b = _ceil4(-(-max(counts[e] for e in small) // 2))
    need_mid = max(counts[e] for e in mid)
    if a + b < need_mid:
        b = _ceil4(need_mid - a)
    # feasibility + profitability vs single-segment
    if 2 * b < max(counts[e] for e in small) or a < 466 or b < 466:
        return None  # segments must each split into >=232-col blocks
    if a + b >= _ceil8(max(counts)):
        return None
    slots_a = [big[0], big[0], big[1], big[1]] + mid
    slots_b = [small[0], small[0], small[1], small[1]] + mid
    return a, b, list(zip(slots_a, slots_b))


def kernel(x, router_w, w1, b1, w2, b2):
    from concourse import bass_utils

    x = np.asarray(x)
    router_w = np.asarray(router_w)
    w1 = np.asarray(w1)
    b1 = np.asarray(b1)
    w2 = np.asarray(w2)
    b2 = np.asarray(b2)

    B, S, _ = x.shape
    T = B * S
    x_flat = x.reshape(T, D)

    idx_list, wgt_list = _route_host(x_flat, router_w)
    counts = [len(i) for i in idx_list]
    plan = _plan_slots(counts)
    if plan is None:
        # degenerate fallback: every core hosts its own expert in both
        # segments (capacity = single-segment capacity, still correct)
        cmax = max(counts)
        a = _ceil8(-(-cmax // 2))
        b = _ceil8(cmax - a)
        plan = (a, b, [(e, e) for e in range(E)])
    a, b, slots = plan

    key = (a, b)
    if key not in _COMPILED:
        _COMPILED[key] = _build_dual_kernel(a, b)
    nc = _COMPILED[key]

    blocks_a, blocks_b = _seg_blocks(a, b)
    blocks = blocks_a + blocks_b
    starts = [sum(blocks[:i]) for i in range(len(blocks))]
    cap = a + b
    bf = ml_dtypes.bfloat16

    # split each expert's tokens greedily across its slots (a-slots first)
    seg_size = {0: a, 1: b}
    slot_tokens = [[None, None] for _ in range(E)]  # per core: [A idx, B idx]
    slot_wgts = [[None, None] for _ in range(E)]
    used = {e: 0 for e in range(E)}
    for seg in range(2):
        for c in range(E):
            e = slots[c][seg]
            s = seg_size[seg]
            lo = used[e]
            hi = min(lo + s, counts[e])
            used[e] = hi
            slot_tokens[c][seg] = idx_list[e][lo:hi]
            slot_wgts[c][seg] = wgt_list[e][lo:hi]
    for e in range(E):
        assert used[e] == counts[e], (e, used[e], counts[e])

    # pre-transpose each expert's weights once; slots share the arrays
    w1_d, w2_d, b1_d = {}, {}, {}
    for e in set(s for pair in slots for s in pair):
        w1_d[e] = np.ascontiguousarray(
            w1[e].reshape(H // P, P, D // P, P).transpose(3, 0, 2, 1)
        ).astype(bf)
        w2_d[e] = np.ascontiguousarray(
            w2[e].reshape(D // P, P, H // P, P).transpose(3, 0, 2, 1)
        ).astype(bf)
        b1_d[e] = np.ascontiguousarray(b1[e].reshape(H // P, P).T).astype(np.float32)

    in_maps = []
    for c in range(E):
        ea, eb = slots[c]
        xg = np.zeros((cap, D), np.float32)
        na = len(slot_tokens[c][0])
        nb_ = len(slot_tokens[c][1])
        xg[:na] = x_flat[slot_tokens[c][0]]
        xg[a : a + nb_] = x_flat[slot_tokens[c][1]]
        xT_full = xg.T.reshape(D // P, P, cap).transpose(1, 0, 2)  # [128, 8, cap]
        xT_d = np.concatenate(
            [xT_full[:, :, st : st + sz].reshape(P, -1) for st, sz in zip(starts, blocks)],
            axis=1,
        ).astype(bf)
        in_maps.append(
            {
                "xT": xT_d,
                "w1A": w1_d[ea], "w2A": w2_d[ea], "b1A": b1_d[ea],
                "w1B": w1_d[eb], "w2B": w2_d[eb], "b1B": b1_d[eb],
            }
        )

    res = bass_utils.run_bass_kernel_spmd(nc, in_maps, core_ids=list(range(E)))

    out = np.zeros((T, D), np.float32)
    for c in range(E):
        ea, eb = slots[c]
        oT = res.results[c]["oT"]  # [128, 8, cap]
        o_g = oT.transpose(1, 0, 2).reshape(D, cap).T  # [cap, D]
        na = len(slot_tokens[c][0])
        nb_ = len(slot_tokens[c][1])
        if na:
            out[slot_tokens[c][0]] += slot_wgts[c][0][:, None] * (
                o_g[:na] + b2[ea][None, :]
            )
        if nb_:
            out[slot_tokens[c][1]] += slot_wgts[c][1][:, None] * (
                o_g[a : a + nb_] + b2[eb][None, :]
            )
    return out.reshape(B, S, D).astype(np.float32)


# revision 39
# speedup vs baseline: 1.0101x; 1.0101x over previous
"""MoE top-2 routing kernel for 8 Trainium2 NeuronCores.

Strategy (expert-parallel with two-segment load balancing):
  - Host computes the (tiny) router in float64: logits -> softmax -> top-2 ->
    renormalize.  Selection was verified tie-safe: min prob gap between
    2nd/3rd expert is ~8e-6 while cross-backend fp32 logit noise is ~3e-7.
  - Plain expert-parallel pads every core to the max expert count (1072 for
    this input vs 1024 average).  Instead each core gets TWO fixed-size
    segments (a, b), each with its own expert weight set; the host assigns
    expert -> slot multisets: the 2 largest experts take two a-slots
    (2a >= c_max), the 2 smallest take two b-slots, the middle four take
    one a + one b (a+b >= c_mid).  For this input a=536, b=500: capacity
    1036/core instead of 1072 (-7.5us of matmul).  Cores stream 2 full
    weight sets (33.6MB, ~150GB/s — well under the ~310GB/s queue budget),
    and all token blocks stay >=232 columns so LDWEIGHTS stays hidden.
  - Each core runs a dense FFN per segment:
        hT = gelu(w1T.T-contractions)   (PSUM fp32 accum, bias fused in ACT)
        oT = w2-contractions over hT
    with D/H features on the partition axis end-to-end (no on-device
    transposes).  Host applies combine weights and scatter-adds outputs.

Schedule notes (from perfetto/ntff trace analysis):
  - The ramp is HBM-bandwidth-bound.  Queue FIFO order IS the schedule:
    sync carries a pure w1 stream in ho-PAIRS (4096B DMA rows; 2048B rows
    measured 2-3x slower per queue), scalar carries the x blocks with b1
    tucked between them (2 DMA queues total; a third gpsimd queue was
    dropped — the ~8.8us teardown epilogue measured identical without it).
  - Fetches not needed until late (B-segment x blocks, early w2 slabs) are
    emitted onto the SYNC queue mid-L1 so queue FIFO places them behind the
    w1 pair stream.  (A memzero-WAR "gate" does NOT work: the scheduler
    hoists the dep-free memzero itself, verified in the trace.)
  - L2 runs segment B first and A last, so the final serial eviction drains
    A's small 240-col block; A splits [296, 240] (not [304, 232]) because a
    232-col matmul (116cy) sits at the LDWEIGHTS boundary (~115cy) and
    jitters LDW-bound.
  - 13 dependency-free warm-up matmuls bridge the ~8.3us fixed prologue so
    the HAM clock-gate is at 8/8 when real matmuls start; steady state runs
    at the bf16 peak (2 cols/cycle).
  - Remaining fixed costs (framework): ~8.3us prologue before the first DMA
    packet, ~8.8us BSP semaphore-teardown epilogue.

Per-core layouts (D=1024, H=4096; cap = a+b tokens, A span [0,a), B [a,cap)):
  xT   [128, 8*cap]        bf16   block-major: xT[p, ko, t] = x_g[t, ko*128+p]
  w1A/B [128, 32, 8, 128]  bf16   w1s[p, ho, ko, j] = w1[e][ho*128+j, ko*128+p]
  w2A/B [128, 8, 32, 128]  bf16   w2s[p, do, ko, j] = w2[e][do*128+j, ko*128+p]
  b1A/B [128, 32]          f32    b1s[p, ho]        = b1[e][ho*128+p]
  oT   [128, 8, cap]       f32    oT[p, do, t]      = o_g[t, do*128+p]
"""

import numpy as np
import ml_dtypes

TOP_K = 2
P = 128
D = 1024
H = 4096
E = 8

_COMPILED = {}  # (a, b) or ('single', C) -> compiled Bacc instance


def _ceil8(n):
    return ((n + 7) // 8) * 8


def _split_seg(S, first):
    """Split segment of S tokens into blocks: `first`-sized lead block, the
    rest as even blocks <=512 (all >=232 when S allows, so LDWEIGHTS stays
    hidden behind the previous matmul)."""
    if S <= 512:
        return [S]
    b0 = min(first, S - 232)
    rem = S - b0
    nblk = -(-rem // 480)
    sizes = [b0]
    for i in range(nblk):
        s = -(-rem // (nblk - i))
        s = min(_ceil8(s), rem)
        sizes.append(s)
        rem -= s
    assert sum(sizes) == S and all(s <= 512 for s in sizes), sizes
    return sizes


def _seg_blocks(a, b):
    # 296/240 rather than 304/232: a 232-col matmul (116cy) sits exactly at
    # the LDWEIGHTS boundary (~115cy) and jitters LDW-bound; 240 has margin.
    return _split_seg(a, 296), _split_seg(b, 272)


def _build_dual_kernel(a, b):
    import concourse.mybir as mybir
    import concourse.tile as tile
    from concourse import bacc

    blocks_a, blocks_b = _seg_blocks(a, b)
    blocks = blocks_a + blocks_b
    seg_of = [0] * len(blocks_a) + [1] * len(blocks_b)
    starts = [sum(blocks[:i]) for i in range(len(blocks))]
    cap = a + b
    NTMAX = max(blocks)
    nb = len(blocks)
    nba = len(blocks_a)
    bf16 = mybir.dt.bfloat16
    f32 = mybir.dt.float32

    nc = bacc.Bacc("TRN2", target_bir_lowering=False, debug=False)
    xT = nc.dram_tensor("xT", [P, D // P * cap], bf16, kind="ExternalInput").ap()
    w1d = [
        nc.dram_tensor(n, [P, H // P, D // P, P], bf16, kind="ExternalInput").ap()
        for n in ("w1A", "w1B")
    ]
    w2d = [
        nc.dram_tensor(n, [P, D // P, H // P, P], bf16, kind="ExternalInput").ap()
        for n in ("w2A", "w2B")
    ]
    b1d = [
        nc.dram_tensor(n, [P, H // P], f32, kind="ExternalInput").ap()
        for n in ("b1A", "b1B")
    ]
    oT = nc.dram_tensor("oT", [P, D // P, cap], f32, kind="ExternalOutput").ap()

    with tile.TileContext(nc) as tc:
        with (
            tc.tile_pool(name="const", bufs=1) as cpool,
            tc.tile_pool(name="resident", bufs=1) as rpool,
            tc.tile_pool(name="warm", bufs=1) as warmpool,
            tc.tile_pool(name="w1p", bufs=6) as w1pool,
            tc.tile_pool(name="w2p", bufs=3) as w2pool,
            tc.tile_pool(name="ost", bufs=4) as opool,
            tc.tile_pool(name="ps", bufs=4, space="PSUM") as pspool,
            tc.tile_pool(name="wps", bufs=1, space="PSUM") as wpspool,
        ):
            # PE warm-up: dependency-free matmuls keep the HAM clock-gate at
            # 8/8 while the first input DMAs are in flight.
            wsrc = warmpool.tile([P, 512], bf16)
            nc.gpsimd.memset(wsrc[:], 0.0)
            wps = wpspool.tile([P, 512], f32)
            for _ in range(13):
                nc.tensor.matmul(wps[:], wsrc[:, :P], wsrc[:], start=True, stop=True)

            # b1 rides the scalar queue (between the x blocks) rather than a
            # dedicated gpsimd queue: one fewer DMA queue to tear down in the
            # fixed epilogue, and the tiny 128B-row descriptors would slow
            # the queue head if placed first.
            b1A_sb = cpool.tile([P, H // P], f32, tag="b1A")
            b1B_sb = cpool.tile([P, H // P], f32, tag="b1B")
            b1_sb = [b1A_sb, b1B_sb]

            w1_tiles = {}

            def fetch_w1(seg, ho):
                base = ho & ~1
                t = w1pool.tile([P, 2, D // P, P], bf16, tag="w1s")
                nc.sync.dma_start(t[:], w1d[seg][:, base : base + 2])
                w1_tiles[(seg, base)] = t
                w1_tiles[(seg, base + 1)] = t

            fetch_w1(0, 0)
            # A-segment x blocks fetch immediately on scalar (needed during
            # the ramp).  B-segment blocks and the early w2A slabs are NOT
            # needed until ~70us/~130us; fetching them early starves the w1
            # pair stream, so their descriptors are emitted onto the SYNC
            # queue mid-L1 — same-queue FIFO places them behind the w1 pairs.
            # (A memzero-WAR "gate" does NOT work: the scheduler hoists the
            # dep-free memzero itself, as seen in the trace.)
            # Only xA0 fetches up front (scalar).  xA1 is needed at lead-end
            # (~18.7us) but fetching it concurrently puts its 0.45MB ahead
            # of pairs 2-3 in GLOBAL bandwidth order — the recurring ~1.7us
            # ramp stall waits on exactly those pairs.  It is instead
            # emitted on the sync queue between pair2 and pair3 (below).
            x_blks = []
            for blk in range(nb):
                st, sz = starts[blk], blocks[blk]
                xb = rpool.tile([P, D // P * sz], bf16, tag=f"xb{blk}")
                if blk == 0:
                    nc.scalar.dma_start(xb[:], xT[:, D // P * st : D // P * (st + sz)])
                    # b1A behind xA0 (needed by the first ACT ~14.5us);
                    # b1B much later (first B ACT ~75us)
                    nc.scalar.dma_start(b1A_sb[:], b1d[0][:])
                x_blks.append(xb)
            nc.scalar.dma_start(b1B_sb[:], b1d[1][:])

            h_sb = rpool.tile([P, H // P, cap], bf16)

            def w1_src(seg, ho, ko):
                return w1_tiles[(seg, ho)][:, ho % 2, ko, :]

            def x_src(blk, ko):
                sz = blocks[blk]
                return x_blks[blk][:, ko * sz : (ko + 1) * sz]

            # Group order: lead = first 6 ho rows on block 0 (later x blocks
            # still in flight), then their remaining A blocks, then ho-major
            # over segment A, then ho-major over segment B (its w1 stream
            # arrives during A's compute).  Keeps slab lifetimes short and
            # slab demand well under delivery after the ramp.
            lead = min(6, H // P) if nba >= 2 else 0
            pairs = [(k, 0) for k in range(lead)]
            for k in range(lead):
                pairs += [(k, bi) for bi in range(1, nba)]
            for ho in range(lead, H // P):
                pairs += [(ho, bi) for bi in range(nba)]
            for ho in range(H // P):
                pairs += [(ho, bi) for bi in range(nba, nb)]

            # Layer 1: hT[:, ho, t] = gelu(sum_ko w1.T @ x + b1)
            w2_early = []
            for pi, (ho, blk) in enumerate(pairs):
                seg = seg_of[blk]
                if (seg, ho) not in w1_tiles:
                    fetch_w1(seg, ho)
                if seg == 0 and blk == 0 and ho == 4:
                    # xA1.. ride sync between pair2 and pair3: arrives
                    # ~17.5us vs needed ~18.7us, without displacing the
                    # stall-critical pairs
                    for bi in range(1, nba):
                        sti, szi = starts[bi], blocks[bi]
                        nc.sync.dma_start(
                            x_blks[bi][:],
                            xT[:, D // P * sti : D // P * (sti + szi)],
                        )
                st, sz = starts[blk], blocks[blk]
                ps = pspool.tile([P, NTMAX], f32, tag="ps")
                for ko in range(D // P):
                    nc.tensor.matmul(
                        ps[:, :sz],
                        w1_src(seg, ho, ko),
                        x_src(blk, ko),
                        start=(ko == 0),
                        stop=(ko == D // P - 1),
                    )
                nc.scalar.activation(
                    h_sb[:, ho, st : st + sz],
                    ps[:, :sz],
                    mybir.ActivationFunctionType.Gelu,
                    bias=b1_sb[seg][:, ho : ho + 1],
                )
                if seg == 0 and blk == 0 and ho == 20:
                    # behind ~10 w1 pairs in sync-queue FIFO; arrives ~30us,
                    # needed at the B sweep (~70us)
                    for bi in range(nb):
                        if seg_of[bi] == 1:
                            sti, szi = starts[bi], blocks[bi]
                            nc.sync.dma_start(
                                x_blks[bi][:],
                                xT[:, D // P * sti : D // P * (sti + szi)],
                            )
                if seg == 0 and blk == 0 and ho in (22, 26, 30):
                    # early w2B slabs (L2 runs segment B first), behind 11-15
                    # w1 pairs in queue order; needed at L2 start (~130us)
                    w2s = w2pool.tile([P, H // P, P], bf16, tag="w2s")
                    nc.sync.dma_start(w2s[:], w2d[1][:, len(w2_early)])
                    w2_early.append(w2s)

            # Layer 2: oT[:, do, t] = sum_ko w2[:,do,ko,:].T @ hT[:,ko,t]
            # Segment B first, A last: the final eviction then drains A's
            # small 240-col block instead of B's 504-col one.
            seg_order = (1, 0)
            for si, seg in enumerate(seg_order):
                sblks = [i for i in range(nb) if seg_of[i] == seg]
                for do in range(D // P):
                    if seg == 1 and do < len(w2_early):
                        w2s = w2_early[do]
                    else:
                        w2s = w2pool.tile([P, H // P, P], bf16, tag="w2s")
                        eng = nc.sync if do % 2 == 0 else nc.scalar
                        eng.dma_start(w2s[:], w2d[seg][:, do])
                    for bi in sblks:
                        st, sz = starts[bi], blocks[bi]
                        ps = pspool.tile([P, NTMAX], f32, tag="ps")
                        for ko in range(H // P):
                            nc.tensor.matmul(
                                ps[:, :sz],
                                w2s[:, ko, :],
                                h_sb[:, ko, st : st + sz],
                                start=(ko == 0),
                                stop=(ko == H // P - 1),
                            )
                        last = (
                            si == len(seg_order) - 1
                            and do == D // P - 1
                            and bi == sblks[-1]
                        )
                        if not last:
                            ob = opool.tile([P, NTMAX], f32, tag="ob")
                            nc.vector.tensor_copy(ob[:, :sz], ps[:, :sz])
                            nc.scalar.dma_start(oT[:, do, st : st + sz], ob[:, :sz])
                        else:
                            # Final eviction is on the critical path: split it
                            # so the first half's DMA overlaps the second
                            # half's copy, using both queues.
                            hsz = sz // 2
                            ob = opool.tile([P, NTMAX], f32, tag="ob")
                            nc.vector.tensor_copy(ob[:, :hsz], ps[:, :hsz])
                            nc.sync.dma_start(oT[:, do, st : st + hsz], ob[:, :hsz])
                            nc.vector.tensor_copy(ob[:, hsz:sz], ps[:, hsz:sz])
                            nc.scalar.dma_start(
                                oT[:, do, st + hsz : st + sz], ob[:, hsz:sz]
                            )

    nc.compile()
    return nc


def _route_host(x_flat, router_w):
    """Float64 router: returns per-expert (token_idx, combine_weight)."""
    logits = x_flat.astype(np.float64) @ router_w.astype(np.float64).T
    m = logits.max(axis=-1, keepdims=True)
    p = np.exp(logits - m)
    p /= p.sum(axis=-1, keepdims=True)
    order = np.argsort(-p, axis=-1)
    topi = order[:, :TOP_K]
    topw = np.take_along_axis(p, topi, axis=-1)
    topw /= topw.sum(axis=-1, keepdims=True)

    idx_list, wgt_list = [], []
    for e in range(E):
        mask = topi == e  # [T, TOP_K]; at most one True per row
        rows = np.nonzero(mask.any(axis=-1))[0]
        w = topw[rows][mask[rows]]
        idx_list.append(rows)
        wgt_list.append(w.astype(np.float32))
    return idx_list, wgt_list


def _plan_slots(counts):
    """Two-segment balancing: returns (a, b, slots) where slots is a list of
    8 (expert_a, expert_b) core assignments, or None if not profitable.
    Each expert's tokens are later split greedily across its slots."""
    def _ceil4(n):
        return ((n + 3) // 4) * 4

    order = sorted(range(E), key=lambda e: -counts[e])
    big, mid, small = order[:2], order[2:-2], order[-2:]
    a = _ceil4(-(-max(counts[e] for e in big) // 2))
    b = _ceil4(-(-max(counts[e] for e in small) // 2))
    need_mid = max(counts[e] for e in mid)
    if a + b < need_mid:
        b = _ceil4(need_mid - a)
    # feasibility + profitability vs single-segment
    if 2 * b < max(counts[e] for e in small) or a < 466 or b < 466:
        return None  # segments must each split into >=232-col blocks
    if a + b >= _ceil8(max(counts)):
        return None
    slots_a = [big[0], big[0], big[1], big[1]] + mid
    slots_b = [small[0], small[0], small[1], small[1]] + mid
    return a, b, list(zip(slots_a, slots_b))


def kernel(x, router_w, w1, b1, w2, b2):
    from concourse import bass_utils

    x = np.asarray(x)
    router_w = np.asarray(router_w)
    w1 = np.asarray(w1)
    b1 = np.asarray(b1)
    w2 = np.asarray(w2)
    b2 = np.asarray(b2)

    B, S, _ = x.shape
    T = B * S
    x_flat = x.reshape(T, D)

    idx_list, wgt_list = _route_host(x_flat, router_w)
    counts = [len(i) for i in idx_list]
    plan = _plan_slots(counts)
    if plan is None:
        # degenerate fallback: every core hosts its own expert in both
        # segments (capacity = single-segment capacity, still correct)
        cmax = max(counts)
        a = _ceil8(-(-cmax // 2))
        b = _ceil8(cmax - a)
        plan = (a, b, [(e, e) for e in range(E)])
    a, b, slots = plan

    key = (a, b)
    if key not in _COMPILED:
        _COMPILED[key] = _build_dual_kernel(a, b)
    nc = _COMPILED[key]

    blocks_a, blocks_b = _seg_blocks(a, b)
    blocks = blocks_a + blocks_b
    starts = [sum(blocks[:i]) for i in range(len(blocks))]
    cap = a + b
    bf = ml_dtypes.bfloat16

    # split each expert's tokens greedily across its slots (a-slots first)
    seg_size = {0: a, 1: b}
    slot_tokens = [[None, None] for _ in range(E)]  # per core: [A idx, B idx]
    slot_wgts = [[None, None] for _ in range(E)]
    used = {e: 0 for e in range(E)}
    for seg in range(2):
        for c in range(E):
            e = slots[c][seg]
            s = seg_size[seg]
            lo = used[e]
            hi = min(lo + s, counts[e])
            used[e] = hi
            slot_tokens[c][seg] = idx_list[e][lo:hi]
            slot_wgts[c][seg] = wgt_list[e][lo:hi]
    for e in range(E):
        assert used[e] == counts[e], (e, used[e], counts[e])

    # pre-transpose each expert's weights once; slots share the arrays
    w1_d, w2_d, b1_d = {}, {}, {}
    for e in set(s for pair in slots for s in pair):
        w1_d[e] = np.ascontiguousarray(
            w1[e].reshape(H // P, P, D // P, P).transpose(3, 0, 2, 1)
        ).astype(bf)
        w2_d[e] = np.ascontiguousarray(
            w2[e].reshape(D // P, P, H // P, P).transpose(3, 0, 2, 1)
        ).astype(bf)
        b1_d[e] = np.ascontiguousarray(b1[e].reshape(H // P, P).T).astype(np.float32)

    in_maps = []
    for c in range(E):
        ea, eb = slots[c]
        xg = np.zeros((cap, D), np.float32)
        na = len(slot_tokens[c][0])
        nb_ = len(slot_tokens[c][1])
        xg[:na] = x_flat[slot_tokens[c][0]]
        xg[a : a + nb_] = x_flat[slot_tokens[c][1]]
        xT_full = xg.T.reshape(D // P, P, cap).transpose(1, 0, 2)  # [128, 8, cap]
        xT_d = np.concatenate(
            [xT_full[:, :, st : st + sz].reshape(P, -1) for st, sz in zip(starts, blocks)],
            axis=1,
        ).astype(bf)
        in_maps.append(
            {
                "xT": xT_d,
                "w1A": w1_d[ea], "w2A": w2_d[ea], "b1A": b1_d[ea],
                "w1B": w1_d[eb], "w2B": w2_d[eb], "b1B": b1_d[eb],
            }
        )

    res = bass_utils.run_bass_kernel_spmd(nc, in_maps, core_ids=list(range(E)))

    out = np.zeros((T, D), np.float32)
    for c in range(E):
        ea, eb = slots[c]
        oT = res.results[c]["oT"]  # [128, 8, cap]
        o_g = oT.transpose(1, 0, 2).reshape(D, cap).T  # [cap, D]
        na = len(slot_tokens[c][0])
        nb_ = len(slot_tokens[c][1])
        if na:
            out[slot_tokens[c][0]] += slot_wgts[c][0][:, None] * (
                o_g[:na] + b2[ea][None, :]
            )
        if nb_:
            out[slot_tokens[c][1]] += slot_wgts[c][1][:, None] * (
                o_g[a : a + nb_] + b2[eb][None, :]
            )
    return out.reshape(B, S, D).astype(np.float32)
